# revision 4
# baseline (speedup 1.0000x reference)
"""Dense dot-product attention with key-length masking on 8 Trainium2 cores.

Problem: q,k,v [16, 2048, 128] fp32, valid_lens [16,1] int32.
  out = softmax(mask(q@k.T/sqrt(d))) @ v   (masked keys -> -1e6 before softmax)

v2 design (vs the group-max baseline):
- Flexible work packing: each core runs the same slot-size profile
  (e.g. [3,14,8,4,3,1]); a slot holds ONE (batch, q-half) unit's key-tile
  RANGE, and a unit may be split across any slots/cores.  The host adds
  the partial numerators/denominators.  This packs the valid_lens-aware
  work almost perfectly (T ~= ceil(total_tiles/8) per core vs 37 for
  sorted group-max), with per-tile masks making padding tiles harmless.
- exp() split across two engines: ACT does `a_j` tiles per slot
  (activation Exp, per-partition scale/bias), DVE does the rest via a
  Schraudolph bit-trick: i16 = round(S*(SCALE*1024*log2e) + 15315)
  bitcast to fp16 ~= exp(S*SCALE) within +-3%.  Masked partitions get
  scale'=0, bias'=0 -> E=+0.0 exactly.
- Softmax denominators: two running fp16 accumulator chains per slot on
  DVE (adds spread through the slot, not bunched at the end); host sums
  the <=2 survivors' 128 partitions and divides.
- Output oT stored fp16 (half the DMA), single [128,1024] copy per slot.
- Input DMA on the SP queue in slot order (slot0 pieces first); output
  DMA kicks on the otherwise-idle Pool queue.  GpSimd does no tensor
  math (SBUF port contention with DVE measured at 3.4x slowdown).
- HAM warm-up: dummy bf16 matmuls while the input DMAs stream.
"""

import math
import sys
import types

import numpy as np

import concourse.bass as bass
import concourse.mybir as mybir
import concourse.tile as tile
from concourse.tile import add_dep_helper
from concourse import bacc
from concourse.bass_utils import run_bass_kernel_spmd

B, Q, K, D = 16, 2048, 2048, 128
NCORES = 8
QCH = 1024         # queries per work unit
KT = K // 128      # max key tiles per unit
MM_N = 512         # moving-operand free dim per matmul
SCALE = 1.0 / math.sqrt(D)
NEG_BIAS = -30.0   # exp(-30) ~ 1e-13: invisible next to real softmax terms
WARMUP_MMS = 7     # dummy matmuls to lift the PE HAM clock-gate
LOG2E = math.log2(math.e)
SCH_MUL = 1024.0 * LOG2E          # Schraudolph fp16 multiplier
SCH_BIAS = 15360.0 - 45.0         # minimax C=45 (DVE rounds): max rel ~3%
ACT_FRAC = 23.0 / 32.0            # fraction of exp tiles on the ACT engine

F32 = mybir.dt.float32
F16 = mybir.dt.float16
BF16 = mybir.dt.bfloat16
I16 = mybir.dt.int16


def _install_hook_stub():
    """bass_utils' axon trace path imports antenv.axon_hooks, which is not
    shipped in this container.  Provide a no-op stub so an ambient
    BASS_TRACE=1 doesn't crash; test harnesses may overwrite the hook."""
    if "antenv.axon_hooks" in sys.modules:
        return
    mod = types.ModuleType("antenv.axon_hooks")
    _hook = [None]
    mod.set_axon_ntff_profile_hook = lambda h: _hook.__setitem__(0, h)
    mod.get_axon_ntff_profile_hook = lambda: _hook[0]
    sys.modules["antenv.axon_hooks"] = mod


_install_hook_stub()

_build_cache = {}
last_result = None  # BassKernelResults of the most recent run (for harnesses)


# ---------------------------------------------------------------- planning

def _pack(profile, units):
    """Greedy: place unit needs (desc) into 8x profile slot inventory,
    splitting freely; returns list of (unit, off, cnt, size_class) pieces
    or None.  Waste = padded tiles."""
    inv = []
    for s in profile:
        inv += [s] * NCORES
    inv.sort(reverse=True)
    pieces = []
    order = sorted(range(len(units)), key=lambda u: -units[u])
    for u in order:
        rem, off = units[u], 0
        while rem > 0:
            if not inv:
                return None
            le = [s for s in inv if s <= rem]
            s = max(le) if le else min(inv)
            inv.remove(s)
            take = min(s, rem)
            pieces.append((u, off, take, s))
            off += take
            rem -= take
    for s in inv:
        pieces.append((None, 0, 0, s))
    return pieces


def _plan(need):
    """need: per-unit key-tile counts (len 32).  Returns (slots, assign):
    slots = slot sizes in execution order; assign[core][slot] =
    (unit | None, off, cnt)."""
    total = int(sum(need))
    tmin = (total + NCORES - 1) // NCORES
    best = None
    from itertools import combinations_with_replacement as cwr
    for T in range(tmin, tmin + 4):
        for m in range(4, 8):
            for prof in cwr(range(1, KT + 1), m):
                if sum(prof) != T:
                    continue
                pieces = _pack(prof, need)
                if pieces is None:
                    continue
                waste = NCORES * T - total
                key = (T, waste, m)
                if best is None or key < best[0]:
                    best = (key, tuple(sorted(prof, reverse=True)), pieces)
        if best is not None and best[0][0] == T:
            break
    prof, pieces = best[1], best[2]
    # execution order: second-smallest first (ignition), then descending,
    # smallest last (tail)
    desc = list(prof)
    slots = [desc[-2]] + desc[:-2] + [desc[-1]] if len(desc) >= 2 else desc
    # distribute pieces of each size class to (core, slot) cells
    cells = {}   # size -> list of (core, slot_idx)
    for j, s in enumerate(slots):
        cells.setdefault(s, [])
        for c in range(NCORES):
            cells[s].append((c, j))
    assign = [[None] * len(slots) for _ in range(NCORES)]
    for (u, off, cnt, s) in pieces:
        c, j = cells[s].pop()
        assign[c][j] = (u, off, cnt)
    for c in range(NCORES):
        for j in range(len(slots)):
            if assign[c][j] is None:
                assign[c][j] = (None, 0, 0)
    return tuple(slots), assign


def _ownership(slots):
    """Per slot: number of leading-interleaved ACT-owned exp tiles.
    DVE-owned tiles are spread through the slot."""
    owner = []
    for t in slots:
        a = int(round(t * ACT_FRAC))
        a = min(t, max(0 if t > 1 else 1, a))
        # dve positions spread evenly: mark which tile indices are DVE
        d = t - a
        dve_pos = set()
        if d > 0:
            for i in range(d):
                dve_pos.add(int((i + 0.5) * t / d))
        owner.append(tuple(i in dve_pos for i in range(t)))
    return tuple(owner)


# ---------------------------------------------------------------- build

def _build(slots, owner):
    nc = bacc.Bacc(num_devices=NCORES)
    NS = len(slots)
    T = sum(slots)

    qT = nc.declare_dram_parameter("qT", [NS, D, QCH], F16, isOutput=False)
    kvs = [
        nc.declare_dram_parameter(f"kv{j}", [128, 2 * slots[j] * 128], F16,
                                  isOutput=False)
        for j in range(NS)
    ]
    # per-slot cols [4*off, 4*off+4*t): [sc_a | bi_a | sc_d | bi_d]
    sb = nc.declare_dram_parameter("sb", [128, 4 * T], F32, isOutput=False)
    oT = nc.declare_dram_parameter("oT", [NS, D, QCH], F16, isOutput=True)
    esum = nc.declare_dram_parameter("esum", [NS, 128, 2 * QCH], F16,
                                     isOutput=True)

    soff = [sum(slots[:j]) for j in range(NS)]

    with tile.TileContext(nc) as tc:
        with (
            tc.tile_pool(name="consts", bufs=1) as consts,
            tc.tile_pool(name="inputs", bufs=2) as inpool,
            tc.tile_pool(name="epool", bufs=max(slots) + 6) as epool,
            tc.tile_pool(name="osb", bufs=2) as opool,
            tc.tile_pool(name="sps", bufs=2, space="PSUM") as pspool,
            tc.tile_pool(name="oacc", bufs=2, space="PSUM") as psacc,
        ):
            # masks for every slot in one small DMA
            sb_sb = consts.tile([128, 4 * T], F32)
            nc.sync.dma_start(out=sb_sb[:], in_=sb[:])

            # --- HAM warm-up: dummy bf16 matmuls while input DMAs stream ---
            wsrc = consts.tile([128, MM_N], BF16)
            nc.vector.memset(wsrc[:], 1.0)
            for w in range(WARMUP_MMS):
                if w % 2 == 0:
                    wps = pspool.tile([128, QCH], F32, tag="s")
                nc.tensor.matmul(
                    wps[:, (w % 2) * MM_N : (w % 2) * MM_N + MM_N],
                    wsrc[:, :128],
                    wsrc[:],
                    start=True,
                    stop=True,
                    skip_group_check=True,
                )

            prev_in_dmas = []
            for s in range(NS):
                t = slots[s]
                ow = owner[s]
                qT_sb = inpool.tile([128, QCH], F16, tag="qT")
                kv_sb = inpool.tile([128, 2 * t * 128], F16, tag="kv")
                in_dmas = []
                nq = 2 if s == 0 else 1
                for j in range(nq):
                    eng = nc.sync if not (s == 0 and j == 1) else nc.scalar
                    d = eng.dma_start(
                        out=qT_sb[:, bass.ts(j, QCH // nq)],
                        in_=qT[s][:, bass.ts(j, QCH // nq)],
                    )
                    in_dmas.append(d)
                # split kv into <=512KB pieces (4KB/partition); slot0's first
                # piece rides the gpsimd queue for an early start
                kvcols = 2 * t * 128
                piece = 2048  # cols per piece = 512KB
                npc = (kvcols + piece - 1) // piece
                for j in range(npc):
                    lo, hi = j * piece, min(kvcols, (j + 1) * piece)
                    keng = nc.gpsimd if (s == 0 and j == 0) else nc.sync
                    in_dmas.append(
                        keng.dma_start(out=kv_sb[:, lo:hi], in_=kvs[s][:, lo:hi])
                    )
                if s == 1:
                    for p in prev_in_dmas:
                        add_dep_helper(
                            in_dmas[0].ins, p.ins,
                            reason="slot0 input DMA priority",
                        )
                prev_in_dmas = in_dmas

                kT_sb = kv_sb[:, : t * 128]

                etiles = []
                o_ps = psacc.tile([128, QCH], F32, tag="o")
                acc = [None, None]  # two running fp16 accumulators
                nacc = 0
                for i in range(t):
                    s_ps = pspool.tile([128, QCH], F32, tag="s")
                    for h in range(QCH // MM_N):
                        nc.tensor.matmul(
                            s_ps[:, bass.ts(h, MM_N)],
                            kT_sb[:, bass.ts(i, 128)],
                            qT_sb[:, bass.ts(h, MM_N)],
                            start=True,
                            stop=True,
                        )
                    col = 4 * soff[s]
                    if ow[i]:
                        # DVE Schraudolph exp -> int16 bitcast fp16
                        ei = epool.tile([128, QCH], I16, tag="e")
                        sc_ap = sb_sb[:, col + 2 * t + i : col + 2 * t + i + 1]
                        bi_ap = sb_sb[:, col + 3 * t + i : col + 3 * t + i + 1]
                        nc.vector.tensor_scalar(
                            ei[:], s_ps[:], sc_ap, bi_ap,
                            mybir.AluOpType.mult, mybir.AluOpType.add,
                        )
                        e_ap = ei[:].bitcast(F16)
                    else:
                        e_sb = epool.tile([128, QCH], F16, tag="e")
                        sc_ap = sb_sb[:, col + i : col + i + 1]
                        bi_ap = sb_sb[:, col + t + i : col + t + i + 1]
                        parts = (
                            [bass.ts(p, MM_N) for p in range(2)]
                            if (s == 0 and i == 0)
                            else [slice(None)]
                        )
                        for pr in parts:
                            nc.scalar.activation(
                                e_sb[:, pr],
                                s_ps[:, pr],
                                mybir.ActivationFunctionType.Exp,
                                bias=bi_ap,
                                scale=sc_ap,
                            )
                        e_ap = e_sb[:]
                    etiles.append(e_ap)
                    v_ap = kv_sb[:, (t + i) * 128 : (t + i + 1) * 128]
                    for h in range(QCH // MM_N):
                        nc.tensor.matmul(
                            o_ps[:, bass.ts(h, MM_N)],
                            v_ap,
                            e_ap[:, bass.ts(h, MM_N)],
                            start=(i == 0),
                            stop=(i == t - 1),
                        )
                    # denominator: two running accumulator chains on DVE
                    if nacc < 2:
                        acc[nacc] = e_ap
                        nacc += 1
                    else:
                        a = i % 2
                        nc.vector.tensor_tensor(
                            acc[a], acc[a], e_ap, mybir.AluOpType.add
                        )

                for a in range(min(nacc, 2)):
                    nc.gpsimd.dma_start(
                        out=esum[s][:, bass.ts(a, QCH)], in_=acc[a]
                    )

                o_sb = opool.tile([128, QCH], F16, tag="osb")
                nc.vector.tensor_copy(o_sb[:], o_ps[:])
                nc.gpsimd.dma_start(out=oT[s], in_=o_sb[:])

    nc.compile()
    return nc


# ---------------------------------------------------------------- host

def kernel(q, k, v, valid_lens):
    q = np.ascontiguousarray(q, dtype=np.float32)
    k = np.ascontiguousarray(k, dtype=np.float32)
    v = np.ascontiguousarray(v, dtype=np.float32)
    L = np.asarray(valid_lens).reshape(-1).astype(np.int64)

    # per-batch key-tile need; L==0 must cover all keys (uniform softmax)
    need_b = np.where(L == 0, KT, np.minimum(KT, (L + 127) // 128)).astype(np.int64)
    units = [(b, h) for b in range(B) for h in range(Q // QCH)]
    need = [int(need_b[b]) for b, h in units]

    slots, assign = _plan(need)
    owner = _ownership(slots)

    key = (slots, owner)
    if key not in _build_cache:
        _build_cache[key] = _build(slots, owner)
    nc = _build_cache[key]

    qh = q.astype(np.float16)
    kh = k.astype(np.float16)
    vh = v.astype(np.float16)

    NS = len(slots)
    T = sum(slots)
    soff = [sum(slots[:j]) for j in range(NS)]

    in_maps = []
    for c in range(NCORES):
        qT_arr = np.zeros((NS, D, QCH), np.float16)
        sb_arr = np.zeros((128, 4 * T), np.float32)
        im = {"qT": qT_arr, "sb": sb_arr}
        for j in range(NS):
            t = slots[j]
            kv = np.zeros((128, 2 * t * 128), np.float16)
            u, off, cnt = assign[c][j]
            col = 4 * soff[j]
            if u is not None and cnt > 0:
                b, h = units[u]
                lb = int(L[b])
                qT_arr[j] = qh[b, h * QCH : (h + 1) * QCH].T
                ksl = kh[b, off * 128 : (off + cnt) * 128]   # [cnt*128, 128]
                kv[:, : cnt * 128] = ksl.T
                vsl = vh[b, off * 128 : (off + cnt) * 128]
                kv[:, t * 128 : (t + cnt) * 128] = (
                    vsl.reshape(cnt, 128, 128).transpose(1, 0, 2).reshape(128, cnt * 128)
                )
                # per-tile masks
                kidx = np.arange(128)
                for i in range(cnt):
                    base = (off + i) * 128
                    if lb == 0:
                        m = None  # uniform: exp(0)=1 on both engines
                        sb_arr[:, col + i] = 0.0
                        sb_arr[:, col + t + i] = 0.0
                        sb_arr[:, col + 2 * t + i] = 0.0
                        sb_arr[:, col + 3 * t + i] = 15360.0
                    else:
                        m = (base + kidx < lb).astype(np.float32)
                        sb_arr[:, col + i] = m * np.float32(SCALE)
                        sb_arr[:, col + t + i] = (1.0 - m) * np.float32(NEG_BIAS)
                        sb_arr[:, col + 2 * t + i] = m * np.float32(SCALE * SCH_MUL)
                        sb_arr[:, col + 3 * t + i] = m * np.float32(SCH_BIAS)
            # padding tiles i in [cnt, t): masks already zero ->
            #   ACT: exp(0*S + 0) = 1?? NO: bias must be NEG for ACT tiles
            for i in range(cnt, t):
                sb_arr[:, col + t + i] = np.float32(NEG_BIAS)  # ACT: exp(-30)
                # DVE: sc=0, bi=0 -> +0.0 exactly
            im[f"kv{j}"] = kv
        in_maps.append(im)

    res = run_bass_kernel_spmd(nc, in_maps, list(range(NCORES)))
    global last_result
    last_result = res

    num = [np.zeros((D, QCH), np.float32) for _ in range(len(units))]
    den = [np.zeros((QCH,), np.float32) for _ in range(len(units))]
    for c in range(NCORES):
        r = res.results[c]
        for j in range(NS):
            u, off, cnt = assign[c][j]
            if u is None or cnt == 0:
                continue
            num[u] += r["oT"][j].astype(np.float32)
            nsur = min(slots[j], 2)
            es = r["esum"][j][:, : nsur * QCH].astype(np.float32)
            den[u] += es.reshape(128, nsur, QCH).sum(axis=(0, 1))

    out = np.empty((B, Q, D), np.float32)
    for ui, (b, h) in enumerate(units):
        out[b, h * QCH : (h + 1) * QCH] = (num[ui] / den[ui][None, :]).T
    return out
